# revision 83
# baseline (speedup 1.0000x reference)
"""Trainium2 Bass kernel for Angles2Backbone (NeRF chain forward).

Full inputs: input [256,3,512] f32, param [6] f32, angles_length [256] i32.
Output: [256, 4608] f32  (coords of 1536 backbone atoms x 3, masked).

Sharding: pure data parallel over batch - 32 proteins per core x 8 cores.

Per-core algorithm (v3, fp16 tree + host-precomputed scalars):
  - Layout: 128 partitions = (3 - chain-quarter)*32 + protein b; each row
    owns 128 consecutive residues (384 atoms), 2-level bit-reversed
    (residue 4j+2b1+b0 at col 64*b0+32*b1+j) so every tree level reads
    and writes contiguous column ranges (fp16 2x DVE mode).
  - Host precomputes the 24 param-derived trig scalars (broadcast table)
    and per-row mask thresholds; device does only the per-residue trig.
  - Pre-pass: D = K(kN)@B_CA via 8 tensor-scalar ops; C1 = Rx(aN)@D via
    6 fused TT ops; Rres = (C1@Rx(aC))@K(kC) similarly. All fp16.
  - Rotation prefixes: pair -> quad products, then Hillis-Steele over 32
    quad mats (5 fused steps, 9 entry planes, fp16 2x).
  - Cross-quarter fixup: PE gathers quarter-end mats via fp16 selector
    matmuls; tiny 3x3 chains on Pool; per-partition frame broadcast via
    PE (quarter order flipped so all PSUM writes hit legal bases).
  - Atom translations: expansion writes a (group, elem, quad) layout;
    one DVE tensor-scalar op per coord permutes to chain order AND emits
    the quarter-end sum via accum_out; single gated tensor_tensor_scan
    over all 3 coord planes (gate=0 resets the chain at plane borders).
  - Translation fixup: quarter sums are already row-aligned with Fbc, so
    3 DVE ops form the global per-quarter translations and ONE PE matmul
    with a strict-lower-triangular (mod-32) selector emits every row's
    quarter-prefix sum directly (chain-q0 rows naturally zero).
  - Final: per-coordinate frame-fix -> masked store -> own-queue DMA
    (c2's products on ACT); fp16 output, widened to f32 on host.
"""

import sys

sys.path.insert(0, "/opt/trn_rl_repo")

import numpy as np
import concourse.bass as bass
import concourse.bacc as bacc
import concourse.mybir as mybir
from concourse import tile
from concourse.bass_utils import run_bass_kernel_spmd

F32 = mybir.dt.float32
F16 = mybir.dt.float16
I32 = mybir.dt.int32
AF = mybir.ActivationFunctionType
OP = mybir.AluOpType

NCORES = 8
BPC = 32          # proteins per core
L = 512           # residues per protein
QN = 4            # chain quarters per protein (partition groups)
W = 384           # atoms per quarter
NR = 128          # residues per quarter (scan length)
PI = float(np.pi)

_CACHE = {}


def _e(i, k):
    return 3 * i + k


def _patch_act_tables():
    # Every activation func this kernel uses (sin, square, identity, copy)
    # lives in the 'trig_and_small' set; putting it first makes the
    # first-match chooser assign one set for all ops -> a single
    # LoadActFuncSet instead of two back-to-back loads on the ACT path.
    # DISABLED: reordering the table list changes act_func_set_id
    # numbering at the BIR level, but walrus codegen resolves IDs against
    # its own act_info.json -- the Sin table would be wrong on hardware.
    return


def _build_graph():
    _patch_act_tables()
    nc = bacc.Bacc("TRN2", target_bir_lowering=False, debug=False,
                   num_devices=NCORES)
    inp = nc.dram_tensor("input", [QN * BPC, 3 * NR], F32,
                     kind="ExternalInput").ap()
    vbt = nc.dram_tensor("vbtab", [128, 24], F32, kind="ExternalInput").ap()
    thrv = nc.dram_tensor("thrv", [128, 1], F32, kind="ExternalInput").ap()
    out = nc.dram_tensor("out", [QN * BPC, 3 * W], F16,
                     kind="ExternalOutput").ap()

    with tile.TileContext(nc) as tc:
        _emit(nc, tc, inp, vbt, thrv, out)
    nc.compile()
    return nc


def _emit(nc, tc, inp, vbt, thrv, out):
    import contextlib
    ctx = contextlib.ExitStack()
    with ctx:
        main = ctx.enter_context(tc.tile_pool(name="main", bufs=1))
        tmps = ctx.enter_context(tc.tile_pool(name="tmps", bufs=24))
        psum = ctx.enter_context(tc.tile_pool(name="psum", bufs=1,
                                              space="PSUM"))

        # ---------------- persistent tiles ----------------
        alpha = main.tile([128, W], F32, tag="alpha")
        ca = main.tile([128, W], F16, tag="ca")
        sa = main.tile([128, W], F16, tag="sa")
        C1 = main.tile([128, 9 * NR], F16, tag="C1")
        RA = main.tile([128, 9 * NR], F16, tag="RA")   # residue mats / Pfull
        RB = main.tile([128, 9 * NR], F16, tag="RB")   # spare temp bank
        QQ = main.tile([128, 6 * NR], F16, tag="QQ")   # q1_i, q2_i
        Vm = main.tile([128, 9 * NR], F16, tag="Vm")   # v1,v2,v3 x 3 coords
        sgate = main.tile([128, 3 * W], F16, tag="sgate")
        ones = main.tile([128, NR], F16, tag="ones")
        Pall = main.tile([128, 3 * W], F16, tag="Pall")
        Pmall = main.tile([128, 3 * W], F16, tag="Pmall")
        jplane_i = main.tile([128, W], I32, tag="jplane_i")
        jplane = main.tile([128, W], F16, tag="jplane")
        maskp = main.tile([128, W], F16, tag="maskp")
        thr = main.tile([128, 1], F32, tag="thr")
        NSC = 24
        Vb = main.tile([128, NSC], F32, tag="Vb")
        Estack = main.tile([BPC, 36], F32, tag="Estack")
        Fstack = main.tile([BPC, 27], F32, tag="Fstack")
        Fbc = main.tile([128, 9], F32, tag="Fbc")
        pestage = main.tile([BPC, 9], F32, tag="pestage")
        cumst = main.tile([BPC, 9], F32, tag="cumst")
        Pincb = main.tile([128, 3], F32, tag="Pincb")
        zb128 = main.tile([128, 1], F32, tag="zb128")

        _cnt = [0]

        def ENG():
            # TT ops only: alternate DVE (2/3) and Pool (1/3)
            _cnt[0] += 1
            return nc.gpsimd if (_cnt[0] % 3 == 0) else nc.vector

        # ---------------- input DMAs ----------------
        # Scalar table (trig of the 6 geometry params, pre-broadcast to all
        # partitions) and mask thresholds are computed host-side: they kill
        # a long serial param->sin->products->broadcast chain at the head.
        # slot layout in Vb[128, NSC]:
        # 0:ckN 1:skN 2:ckA 3:skA 4:ckC 5:skC
        # 6:ckNckA 7:skNskA 8:ckNskA 9:skNckA
        # 10:nskNckA 11:nckNskA 12:nckN 13:nckA 14:nckC 15:nskA
        # 16:RNckN 17:RNskN 18:RCA 19:RC
        # all three input DMAs go on the SP queue: issuing any of them from
        # the scalar queue would push the ACT table loads (which gate the
        # first sin) back by the 667ns issue cost each.
        av = alpha[:]
        nc.sync.dma_start(av[:, 0:NR], inp[:, 0:NR])        # type A block
        nc.sync.dma_start(av[:, NR:3 * NR], inp[:, NR:3 * NR])
        nc.sync.dma_start(Vb[:], vbt[:])
        nc.sync.dma_start(thr[:], thrv[:])
        # dependency-free dummy activation: the ACT table load is inserted
        # before the first activation and inherits its waits, so give it a
        # wait-free anchor -> the 1.3us load overlaps the input DMA instead
        # of following it. (Reads uninitialized scratch; output unused.)
        warm = main.tile([128, 1], F32, tag="warm")
        nc.scalar.activation(warm[:], warm[:], AF.Sin, bias=zb128[:])
        nc.vector.memset(zb128[:], 0.0)

        # selector matrices for PE-based cross-partition gather/broadcast
        rowid_i = main.tile([128, 1], I32, tag="rowid_i")
        rowid = main.tile([128, 1], F32, tag="rowid")
        colid = main.tile([128, 32], I32, tag="colid")
        rowq = main.tile([128, 1], F32, tag="rowq")
        I32f = main.tile([BPC, BPC], F32, tag="I32f")
        selq = main.tile([128, 4 * BPC], F16, tag="selq")
        tri = main.tile([128, 128], F32, tag="tri")
        trit = main.tile([128, 128], F32, tag="trit")
        colid128 = main.tile([128, 128], I32, tag="colid128")
        nc.gpsimd.iota(rowid_i[:], [[0, 1]], channel_multiplier=1)
        nc.gpsimd.iota(colid[:], [[1, BPC]], channel_multiplier=0)
        nc.vector.tensor_copy(rowid[:], rowid_i[:])
        nc.vector.tensor_scalar(I32f[0:BPC, 0:BPC], colid[0:BPC, :],
                                rowid[0:BPC, 0:1], None, op0=OP.is_equal)
        for q in range(QN):
            # chain quarter q lives at partition rows (3-q)*32 so that the
            # PE fixup broadcasts for q=1..3 land at legal bases 64/32/0.
            nc.vector.tensor_scalar(rowq[:], rowid[:],
                                    float((3 - q) * BPC), None,
                                    op0=OP.subtract)
            nc.vector.tensor_scalar(selq[:, q * BPC:(q + 1) * BPC], colid[:],
                                    rowq[:, 0:1], None, op0=OP.is_equal)

        # strict-lower-triangular (mod-32) selector: tri[k,m]=1 iff
        # k-m in {32,64,96}
        nc.gpsimd.iota(colid128[:], [[1, 128]], channel_multiplier=0)
        for di, d in enumerate((32.0, 64.0, 96.0)):
            nc.vector.tensor_scalar(rowq[:], rowid[:], d, None,
                                    op0=OP.subtract)
            dst = tri if di == 0 else trit
            nc.vector.tensor_scalar(dst[:, 0:128], colid128[:],
                                    rowq[:, 0:1], None, op0=OP.is_equal)
            if di > 0:
                nc.vector.tensor_add(tri[:, 0:128], tri[:, 0:128],
                                     trit[:, 0:128])

        PSg = psum.tile([BPC, 36], F32, tag="PSg")
        PSf = psum.tile([128, 9], F32, tag="PSf")
        PSp = psum.tile([BPC, 9], F32, tag="PSp")
        PSi = psum.tile([128, 3], F32, tag="PSi")

        S = {}
        for i, nm in enumerate(("ckN", "skN", "ckA", "skA", "ckC", "skC",
                                "ckNckA", "skNskA", "ckNskA", "skNckA",
                                "nskNckA", "nckNskA", "nckN", "nckA",
                                "nckC", "nskA", "RNckN", "RNskN",
                                "RCA", "RC")):
            S[nm] = Vb[:, i:i + 1]

        # trig: |alpha| < 4pi: s4=sin(a/4), c4=1-2sin^2(a/8);
        # s2=2*s4*c4, c2=1-2*s4^2; s1=2*s2*c2, c1=1-2*s2^2.
        # One chain per 128-col type block, pipelined across ACT/DVE.
        def trig_chain(bs):
            # double-angle chain: sin/cos(alpha) from sin(alpha/8)
            nb = bs.stop - bs.start
            avb, cab, sab = av[:, bs], ca[:, bs], sa[:, bs]
            ts8 = tmps.tile([128, nb], F32, tag="t1")
            ts4 = tmps.tile([128, nb], F32, tag="t2")
            tq = tmps.tile([128, nb], F32, tag="t1")
            nc.scalar.activation(ts8[:], avb, AF.Sin, bias=zb128[:],
                                 scale=0.125)
            nc.scalar.activation(ts4[:], avb, AF.Sin, bias=zb128[:],
                                 scale=0.25)
            nc.scalar.square(ts8[:], ts8[:])
            nc.vector.tensor_scalar(cab, ts8[:], -2.0, 1.0,
                                    op0=OP.mult, op1=OP.add)          # c4
            nc.vector.scalar_tensor_tensor(ts8[:], ts4[:], 2.0, cab,
                                           op0=OP.mult, op1=OP.mult)  # s2
            nc.scalar.square(tq[:], ts4[:])
            nc.vector.tensor_scalar(ts4[:], tq[:], -2.0, 1.0,
                                    op0=OP.mult, op1=OP.add)          # c2
            nc.vector.scalar_tensor_tensor(sab, ts8[:], 2.0, ts4[:],
                                           op0=OP.mult, op1=OP.mult)  # s1
            nc.scalar.square(tq[:], ts8[:])
            nc.vector.tensor_scalar(cab, tq[:], -2.0, 1.0,
                                    op0=OP.mult, op1=OP.add)          # c1

        # per-type cos/sin views [128, 128]; host packs types (A, N, C)
        cA, sA = ca[:, 0:128], sa[:, 0:128]
        cN, sN = ca[:, 128:256], sa[:, 128:256]
        cC, sC = ca[:, 256:384], sa[:, 256:384]

        def blk(t, e, lo=0, hi=NR):
            return t[:, e * NR + lo:e * NR + hi]

        V = nc.vector
        STT = nc.vector.scalar_tensor_tensor
        TS = nc.vector.tensor_scalar

        def ap3(base_ap, off, dims):
            return bass.AP(base_ap.tensor, base_ap.offset + off,
                           [list(base_ap.ap[0])] + [list(d) for d in dims])

        c1 = C1[:]
        qq = QQ[:]
        vm = Vm[:]
        ra = RA[:]

        trig_chain(slice(0, NR))     # type A: gates the D-build

        # D = K(kN) @ B_CA depends only on type-A trig: emit before the
        # N+C chain so DVE's in-order queue doesn't stall on their sins.
        TS(blk(c1, 0), cA, S["skNskA"], S["ckNckA"],
           op0=OP.mult, op1=OP.add)                        # D00 = C1_00
        TS(blk(c1, 1), cA, S["nskNckA"], S["ckNskA"],
           op0=OP.mult, op1=OP.add)                        # D01 = C1_01
        V.tensor_scalar_mul(blk(c1, 2), sA, S["skN"])      # D02 = C1_02
        TS(blk(qq, 0), cA, S["nckNskA"], S["skNckA"],
           op0=OP.mult, op1=OP.add)                        # D10
        TS(blk(qq, 1), cA, S["ckNckA"], S["skNskA"],
           op0=OP.mult, op1=OP.add)                        # D11
        V.tensor_scalar_mul(blk(qq, 2), sA, S["nckN"])     # D12
        V.tensor_scalar_mul(blk(qq, 3), sA, S["nskA"])     # D20
        V.tensor_scalar_mul(blk(qq, 4), sA, S["ckA"])      # D21
        V.tensor_copy(blk(qq, 5), cA)                      # D22

        trig_chain(slice(NR, 3 * NR))   # types N + C in one wide chain
        nc.gpsimd.memset(ones[:], 1.0)
        nc.gpsimd.memset(sgate[:], 1.0)
        for c in range(3):
            nc.gpsimd.memset(sgate[:, c * W:c * W + 1], 0.0)


        nc.gpsimd.iota(jplane_i[:], [[1, W]], channel_multiplier=0)
        nc.vector.tensor_copy(jplane[:], jplane_i[:])
        nc.vector.tensor_scalar(maskp[:], jplane[:], thr[:, 0:1], None,
                                op0=OP.is_lt)

        # ------------- pre-pass: C1 = B_N @ B_CA = Rx(aN) @ D -------------
        # D = K(kN) @ B_CA has entries affine in (cA, sA) with per-type
        # scalar coefficients -> 8 TS(4x) ops + 1 copy. Row 0 of D IS row 0
        # of C1; rows 1,2 compose via Rx(aN) with 6 fused TT(2x) ops.

        def rowap(base, p0, n=3):
            return ap3(base, p0 * NR, [[NR, n], [1, NR]])

        def rowaps(base, p0, step, n=3):
            return ap3(base, p0 * NR, [[step * NR, n], [1, NR]])

        def bcast(plane):
            return ap3(plane, 0, [[0, 3], [1, NR]])

        TA = main.tile([128, 6 * NR], F16, tag="TA")
        TB = main.tile([128, 6 * NR], F16, tag="TB")
        # C1 row1 = cN*D1 - sN*D2 ; row2 = sN*D1 + cN*D2
        V.tensor_mul(rowap(TA[:], 0), bcast(cN), rowap(qq, 0))
        V.tensor_mul(rowap(TB[:], 0), bcast(sN), rowap(qq, 3))
        V.tensor_mul(rowap(TA[:], 3), bcast(sN), rowap(qq, 0))
        V.tensor_mul(rowap(TB[:], 3), bcast(cN), rowap(qq, 3))
        V.tensor_sub(rowap(c1, 3), rowap(TA[:], 0), rowap(TB[:], 0))
        V.tensor_add(rowap(c1, 6), rowap(TA[:], 3), rowap(TB[:], 3))

        # residue-0 of q=0: B_N := Identity => C1 := B_CA(0)
        # (alpha_CA(0)=0 so cA=1, sA=0 there): [[ckA,skA,0],[skA,-ckA,0],
        # [0,0,-1]]
        r0s = slice(3 * BPC, 128)   # chain q=0 rows
        o1 = ones[r0s, 0:1]
        V.tensor_scalar_mul(c1[r0s, 0 * NR:0 * NR + 1], o1, S["ckA"][r0s])
        V.tensor_scalar_mul(c1[r0s, 1 * NR:1 * NR + 1], o1, S["skA"][r0s])
        nc.gpsimd.memset(c1[r0s, 2 * NR:2 * NR + 1], 0.0)
        V.tensor_scalar_mul(c1[r0s, 3 * NR:3 * NR + 1], o1, S["skA"][r0s])
        V.tensor_scalar_mul(c1[r0s, 4 * NR:4 * NR + 1], o1, S["nckA"][r0s])
        nc.gpsimd.memset(c1[r0s, 5 * NR:5 * NR + 1], 0.0)
        nc.gpsimd.memset(c1[r0s, 6 * NR:6 * NR + 1], 0.0)
        nc.gpsimd.memset(c1[r0s, 7 * NR:7 * NR + 1], 0.0)
        nc.gpsimd.memset(c1[r0s, 8 * NR:8 * NR + 1], -1.0)

        # -------- Rres = C1 @ B_C = (C1 @ Rx(aC)) @ K(kC) -> RA ----------
        # G col0 = C1 col0 (aliased); G col1/col2 from cC/sC TT combines;
        # K(kC) applies with per-type scalars (TSmul + STT per column).
        # Gcol1 -> QQ planes 0..2, Gcol2 -> QQ planes 3..5 (D is dead).
        V.tensor_mul(rowap(TA[:], 0), bcast(cC), rowaps(c1, 1, 3))
        V.tensor_mul(rowap(TB[:], 0), bcast(sC), rowaps(c1, 2, 3))
        V.tensor_mul(rowap(TA[:], 3), bcast(sC), rowaps(c1, 1, 3))
        V.tensor_mul(rowap(TB[:], 3), bcast(cC), rowaps(c1, 2, 3))
        V.tensor_add(rowap(qq, 0), rowap(TA[:], 0), rowap(TB[:], 0))
        V.tensor_sub(rowap(qq, 3), rowap(TB[:], 3), rowap(TA[:], 3))
        # K stage: col0 = ckC*C1col0 + skC*Gcol1; col1 = skC*C1col0
        # - ckC*Gcol1; col2 = -Gcol2
        V.tensor_scalar_mul(rowap(TA[:], 0), rowaps(c1, 0, 3), S["ckC"])
        V.tensor_scalar_mul(rowap(TA[:], 3), rowap(qq, 0), S["skC"])
        V.tensor_scalar_mul(rowap(TB[:], 0), rowaps(c1, 0, 3), S["skC"])
        V.tensor_scalar_mul(rowap(TB[:], 3), rowap(qq, 0), S["nckC"])
        V.tensor_add(rowaps(ra, 0, 3), rowap(TA[:], 0), rowap(TA[:], 3))
        V.tensor_add(rowaps(ra, 1, 3), rowap(TB[:], 0), rowap(TB[:], 3))
        V.tensor_scalar_mul(rowaps(ra, 2, 3), rowap(qq, 3), -1.0)

        # ---------------- v-vectors for atom expansion ----------------
        # v1 = t_N = RN*(ckN, skN*cN, skN*sN); v2 = RCA*C1[:,0];
        # v3 = RC*Rres[:,0]
        V.tensor_scalar_mul(blk(vm, 0), ones[:], S["RNckN"])
        V.tensor_scalar_mul(blk(vm, 1), cN, S["RNskN"])
        V.tensor_scalar_mul(blk(vm, 2), sN, S["RNskN"])
        V.tensor_scalar_mul(rowap(vm, 3), rowaps(c1, 0, 3), S["RCA"])
        V.tensor_scalar_mul(rowap(vm, 6), rowaps(ra, 0, 3), S["RC"])

        # ---------------- Hillis-Steele residue scan ----------------
        # Fused step: all 9 output entries in one 3-dim AP op per k-term:
        #   out[i,j] += L[i,k] (bcast over j) * R[k,j] (bcast over i)
        # 5 logical ops per step, each split col-wise DVE/Pool.
        T9a = main.tile([128, 9 * NR], F16, tag="T9a")
        T9b = main.tile([128, 9 * NR], F16, tag="T9b")
        T9c = main.tile([128, 9 * (NR // 2)], F16, tag="T9c")
        T9d = main.tile([128, 9 * (NR // 2)], F16, tag="T9d")

        def fused_step(srcb, dstb, s, nr):
            n = nr - s
            cut = n                     # DVE-only: fp16 2x beats Pool 4x over
            sv = srcb.rearrange("p (e j) -> p e j", e=9)
            dv = dstb.rearrange("p (e j) -> p e j", e=9)
            nc.vector.tensor_copy(dv[:, :, 0:s], sv[:, :, 0:s])

            def L(k, c0, c1):
                return ap3(srcb, k * nr + c0,
                           [[3 * nr, 3], [0, 3], [1, c1 - c0]])

            def R(k, c0, c1):
                return ap3(srcb, 3 * k * nr + s + c0,
                           [[0, 3], [nr, 3], [1, c1 - c0]])

            def T(t, c0, c1):
                return ap3(t[:], c0, [[3 * nr, 3], [nr, 3], [1, c1 - c0]])

            def O(c0, c1):
                return ap3(dstb, s + c0, [[3 * nr, 3], [nr, 3], [1, c1 - c0]])

            for E, c0, c1 in ((nc.vector, 0, cut), (nc.gpsimd, cut, n)):
                if c1 <= c0:
                    continue
                E.tensor_mul(T(T9a, c0, c1), L(0, c0, c1), R(0, c0, c1))
                E.tensor_mul(T(T9b, c0, c1), L(1, c0, c1), R(1, c0, c1))
                E.tensor_mul(T(T9c, c0, c1), L(2, c0, c1), R(2, c0, c1))
                E.tensor_add(T(T9a, c0, c1), T(T9a, c0, c1), T(T9b, c0, c1))
                E.tensor_add(O(c0, c1), T(T9a, c0, c1), T(T9c, c0, c1))

        # pair adjacent residues: P2[r'] = Rres[2r'] @ Rres[2r'+1]
        # Residues arrive 2-level bit-reversed per row: residue 4j+2b1+b0
        # sits at col 64*b0 + 32*b1 + j, so evens = cols 0:64 (pair m at
        # col2(m)=32*(m%2)+m//2), odds = cols 64:128 -- all reads/writes
        # contiguous => fp16 2x DVE mode throughout the tree.
        NR2 = NR // 2
        P2A = main.tile([128, 9 * NR2], F16, tag="P2A")
        P2B = main.tile([128, 9 * NR2], F16, tag="P2B")
        pcut = NR2 - 8
        ra_ap = RA[:]

        def PL(k, c0, c1):
            return ap3(ra_ap, k * NR + c0,
                       [[3 * NR, 3], [0, 3], [1, c1 - c0]])

        def PR(k, c0, c1):
            return ap3(ra_ap, 3 * k * NR + 64 + c0,
                       [[0, 3], [NR, 3], [1, c1 - c0]])

        def PT(t, c0, c1):
            return ap3(t[:], c0, [[3 * NR2, 3], [NR2, 3], [1, c1 - c0]])

        def PO(c0, c1):
            return ap3(P2A[:], c0, [[3 * NR2, 3], [NR2, 3], [1, c1 - c0]])

        for E, c0, c1 in ((nc.vector, 0, pcut), (nc.gpsimd, pcut, NR2)):
            E.tensor_mul(PT(T9a, c0, c1), PL(0, c0, c1), PR(0, c0, c1))
            E.tensor_mul(PT(T9b, c0, c1), PL(1, c0, c1), PR(1, c0, c1))
            E.tensor_mul(PT(T9c, c0, c1), PL(2, c0, c1), PR(2, c0, c1))
            E.tensor_add(PT(T9a, c0, c1), PT(T9a, c0, c1), PT(T9b, c0, c1))
            E.tensor_add(PO(c0, c1), PT(T9a, c0, c1), PT(T9c, c0, c1))

        Wodd = main.tile([128, 9 * NR2], F16, tag="Wodd")
        wo = Wodd[:]
        wcut = NR2 - 8

        def WL(k, c0, c1):
            return ap3(ra_ap, k * NR + c0,
                       [[3 * NR, 3], [0, 3], [1, c1 - c0]])

        def WR(k, c0, c1):
            return ap3(vm, k * NR + 64 + c0,
                       [[0, 3], [3 * NR, 3], [1, c1 - c0]])

        def WT(t, c0, c1):
            return ap3(t[:], c0, [[3 * NR2, 3], [NR2, 3], [1, c1 - c0]])

        def WO(c0, c1):
            return ap3(wo, c0, [[NR2, 3], [3 * NR2, 3], [1, c1 - c0]])

        for E, c0, c1 in ((nc.vector, 0, wcut), (nc.gpsimd, wcut, NR2)):
            E.tensor_mul(WT(T9a, c0, c1), WL(0, c0, c1), WR(0, c0, c1))
            E.tensor_mul(WT(T9b, c0, c1), WL(1, c0, c1), WR(1, c0, c1))
            E.tensor_mul(WT(T9c, c0, c1), WL(2, c0, c1), WR(2, c0, c1))
            E.tensor_add(WT(T9a, c0, c1), WT(T9a, c0, c1), WT(T9b, c0, c1))
            E.tensor_add(WO(c0, c1), WT(T9a, c0, c1), WT(T9c, c0, c1))


        # quad level: P4[r''] = P2[2r''] @ P2[2r''+1]
        NR4 = NR2 // 2
        P4A = main.tile([128, 9 * NR4], F16, tag="P4A")
        P4B = main.tile([128, 9 * NR4], F16, tag="P4B")
        W2 = main.tile([128, 18 * NR4], F16, tag="W2")
        p2a = P2A[:]
        w2 = W2[:]
        qcut = NR4

        def QL(k, c0, c1):
            return ap3(p2a, k * NR2 + c0,
                       [[3 * NR2, 3], [0, 3], [1, c1 - c0]])

        def QR(k, c0, c1):
            return ap3(p2a, 3 * k * NR2 + 32 + c0,
                       [[0, 3], [NR2, 3], [1, c1 - c0]])

        def QT(t, c0, c1):
            return ap3(t[:], c0, [[3 * NR4, 3], [NR4, 3], [1, c1 - c0]])

        def QO(c0, c1):
            return ap3(P4A[:], c0, [[3 * NR4, 3], [NR4, 3], [1, c1 - c0]])

        rb1 = RB[:, 0:9 * NR2]
        rb2 = RB[:, 9 * NR2:18 * NR2]
        for E, c0, c1 in ((nc.vector, 0, NR4),):
            E.tensor_mul(QT(T9a, c0, c1), QL(0, c0, c1), QR(0, c0, c1))
            E.tensor_mul(QT(T9b, c0, c1), QL(1, c0, c1), QR(1, c0, c1))
            E.tensor_mul(QT(rb1, c0, c1), QL(2, c0, c1), QR(2, c0, c1))
            E.tensor_add(QT(T9a, c0, c1), QT(T9a, c0, c1), QT(T9b, c0, c1))
            E.tensor_add(QO(c0, c1), QT(T9a, c0, c1), QT(rb1, c0, c1))

        # W2 group A (m~=0..2): P2_even @ (vm at residues 4r''+2)
        # W2 group B (m~=3..5): P2_even @ (Wodd at odd superblocks)
        def W2L(k, c0, c1):
            return ap3(p2a, k * NR2 + c0,
                       [[3 * NR2, 3], [0, 3], [1, c1 - c0]])

        def W2RA(k, c0, c1):
            return ap3(vm, k * NR + 32 + c0,
                       [[0, 3], [3 * NR, 3], [1, c1 - c0]])

        def W2RB(k, c0, c1):
            return ap3(wo, k * NR2 + 32 + c0,
                       [[0, 3], [3 * NR2, 3], [1, c1 - c0]])

        def W2O(goff, c0, c1):
            return ap3(w2, goff + c0, [[NR4, 3], [3 * NR4, 3], [1, c1 - c0]])

        for goff, RF in ((0, W2RA), (9 * NR4, W2RB)):
            for E, c0, c1 in ((nc.vector, 0, NR4),):
                E.tensor_mul(QT(T9c, c0, c1), W2L(0, c0, c1), RF(0, c0, c1))
                E.tensor_mul(QT(T9d, c0, c1), W2L(1, c0, c1), RF(1, c0, c1))
                E.tensor_mul(QT(rb2, c0, c1), W2L(2, c0, c1), RF(2, c0, c1))
                E.tensor_add(QT(T9c, c0, c1), QT(T9c, c0, c1),
                             QT(T9d, c0, c1))
                E.tensor_add(W2O(goff, c0, c1), QT(T9c, c0, c1),
                             QT(rb2, c0, c1))

        bufs = [P4A, P4B]
        nsteps = 5
        for step in range(nsteps):
            fused_step(bufs[step % 2][:], bufs[(step + 1) % 2][:],
                       1 << step, NR4)
        Rscan = bufs[nsteps % 2][:]    # local quad prefixes, sequential

        # ---------------- cross-quarter rotation fixup ----------------
        for q in range(QN):
            nc.tensor.matmul(
                PSg[0:BPC, q * 9:(q + 1) * 9],
                selq[:, q * BPC:(q + 1) * BPC],
                Rscan[:, NR4 - 1:9 * NR4:NR4], start=True, stop=True)
        # tiny 3x3 chain runs on Pool/ACT so DVE's in-order queue flows
        # straight from the HS scan into the atom expansion.
        nc.scalar.copy(Estack[0:BPC, 0:36], PSg[0:BPC, 0:36])
        nc.scalar.copy(Fstack[0:BPC, 0:9], Estack[0:BPC, 0:9])
        mt0 = main.tile([BPC, 9], F32, tag="mt0")
        mt1 = main.tile([BPC, 9], F32, tag="mt1")
        fs = Fstack[:]
        es = Estack[:]

        def ap2(base_ap, off, dims):
            return bass.AP(base_ap.tensor, base_ap.offset + off,
                           [list(base_ap.ap[0])] + [list(d) for d in dims])

        G = nc.gpsimd
        for q in (1, 2):
            FL = lambda k: ap2(fs, (q - 1) * 9 + k, [[3, 3], [0, 3]])
            ER = lambda k: ap2(es, q * 9 + 3 * k, [[0, 3], [1, 3]])
            MT = lambda t: ap2(t[:], 0, [[3, 3], [1, 3]])
            FO = ap2(fs, q * 9, [[3, 3], [1, 3]])
            G.tensor_mul(MT(mt0), FL(0), ER(0))
            G.tensor_mul(MT(mt1), FL(1), ER(1))
            G.tensor_add(MT(mt0), MT(mt0), MT(mt1))
            G.tensor_mul(MT(mt1), FL(2), ER(2))
            G.tensor_add(FO, MT(mt0), MT(mt1))
        nc.gpsimd.memset(Fbc[3 * BPC:128, 0:9], 0.0)
        for e in (0, 4, 8):
            nc.gpsimd.memset(Fbc[3 * BPC:128, e:e + 1], 1.0)
        for q in (1, 2, 3):
            rb = (3 - q) * BPC
            nc.tensor.matmul(
                PSf[rb:rb + BPC, 0:9], I32f[0:BPC, 0:BPC],
                Fstack[0:BPC, (q - 1) * 9:q * 9], start=True, stop=True)
        for q in (1, 2, 3):
            rb = (3 - q) * BPC
            nc.scalar.copy(Fbc[rb:rb + BPC, 0:9],
                           PSf[rb:rb + BPC, 0:9])
        # ---------------- atom translations (local frame) ----------------
        # superblock = 2 residues = 6 atoms. w_m (m=0..5): prefix-within-
        # superblock applied to t-vectors; m<3 are the per-residue v's at
        # even residues, m>=3 need Rres_even @ v_odd (fused below).
        # Uloc layout per coord plane: col = g*96 + k*32 + t  (group g,
        # element k, quad t) so expansion writes are t-contiguous (2x mode);
        # the prefix scan walks chain order via a strided 3-dim AP.
        Uloc = main.tile([128, 3 * W], F16, tag="Uloc")
        ul = Uloc[:]
        rs = Rscan
        # quad 0 (t=0): local prefix = identity -> u = w_m
        nc.scalar.copy(ap3(ul, 0, [[W, 3], [32, 3]]),
                       ap3(vm, 0, [[NR, 3], [3 * NR, 3]]))
        nc.scalar.copy(ap3(ul, 96, [[W, 3], [32, 3]]),
                       ap3(wo, 0, [[NR2, 3], [3 * NR2, 3]]))
        nc.scalar.copy(ap3(ul, 192, [[W, 3], [32, 3]]),
                       ap3(w2, 0, [[NR4, 3], [3 * NR4, 3]]))
        nc.scalar.copy(ap3(ul, 288, [[W, 3], [32, 3]]),
                       ap3(w2, 9 * NR4, [[NR4, 3], [3 * NR4, 3]]))
        nu = NR4 - 1
        ucut = nu

        def UL(k, c0, c1):
            return ap3(rs, k * NR4 + c0, [[3 * NR4, 3], [0, 3], [1, c1 - c0]])

        def UR0(k, c0, c1):   # atoms 12t+0..2: vm at residue 4t (col t)
            return ap3(vm, k * NR + 1 + c0,
                       [[0, 3], [3 * NR, 3], [1, c1 - c0]])

        def UR1(k, c0, c1):   # atoms +3..5: Wodd at even pair (col t)
            return ap3(wo, k * NR2 + 1 + c0,
                       [[0, 3], [3 * NR2, 3], [1, c1 - c0]])

        def UR2(k, c0, c1):   # atoms +6..8: W2 group A
            return ap3(w2, k * NR4 + 1 + c0,
                       [[0, 3], [3 * NR4, 3], [1, c1 - c0]])

        def UR3(k, c0, c1):   # atoms +9..11: W2 group B
            return ap3(w2, 9 * NR4 + k * NR4 + 1 + c0,
                       [[0, 3], [3 * NR4, 3], [1, c1 - c0]])

        def UT(t, c0, c1):
            return ap3(t[:], c0, [[3 * NR4, 3], [NR4, 3], [1, c1 - c0]])

        def UO(off, c0, c1):
            return ap3(ul, off + 1 + c0, [[W, 3], [32, 3], [1, c1 - c0]])

        for gi, (off, RF) in enumerate(((0, UR0), (96, UR1),
                                        (192, UR2), (288, UR3))):
            ta = [T9a, T9c][gi % 2]
            tb = [T9b, T9d][gi % 2]
            tc3 = [rb1, rb2][gi % 2]
            for E, c0, c1 in ((nc.vector, 0, nu),):
                E.tensor_mul(UT(ta, c0, c1), UL(0, c0, c1), RF(0, c0, c1))
                E.tensor_mul(UT(tb, c0, c1), UL(1, c0, c1), RF(1, c0, c1))
                E.tensor_mul(UT(tc3, c0, c1), UL(2, c0, c1), RF(2, c0, c1))
                E.tensor_add(UT(ta, c0, c1), UT(ta, c0, c1), UT(tb, c0, c1))
                E.tensor_add(UO(off, c0, c1), UT(ta, c0, c1), UT(tc3, c0, c1))
        # prefix-sum the LOCAL u per coordinate in CHAIN order (frame fix
        # applied at the end by linearity: sum_j F@u = F@sum_j u)
        pm = Pmall[:]
        for c in range(3):
            V.memset(ul[3 * BPC:128, c * W:c * W + 1], 0.0)  # chain atom 0

        # permute (g,k,t) -> chain order (Pmall as scratch) AND produce the
        # quarter-end translation (= full-row sum) via accum_out in the same
        # DVE op; the fixup consumes PEnd32 directly via the F32 selector.
        PEnd32 = main.tile([128, 3], F32, tag="PEnd32")
        for c in range(3):
            nc.vector.tensor_scalar(
                ap3(pm, c * W, [[12, 32], [3, 4], [1, 3]]),
                ap3(ul, c * W, [[1, 32], [96, 4], [32, 3]]),
                1.0, 0.0, op0=OP.mult, op1=OP.add,
                accum_out=PEnd32[:, c:c + 1])
        # single fused scan over all 3 coordinate planes: state = g*state + u
        # with gate g = 0 at each plane's first column (resets the chain)
        nc.vector.tensor_tensor_scan(
            Pall[:, 0:3 * W], sgate[:, 0:3 * W], pm[:, 0:3 * W],
            0.0, op0=OP.mult, op1=OP.add)

        # ---------------- cross-quarter translation fixup ----------------
        # PEnd32's rows already align with Fbc's per-row frames: global
        # per-quarter translations = per-partition 3x3 matvec (3 DVE ops);
        # then ONE PE matmul with a strict-lower-triangular (mod-32)
        # selector yields every row's quarter-prefix sum (chain-q0 rows
        # are naturally zero).
        pv = Pall[:]
        prod9 = main.tile([128, 9], F32, tag="prod9")
        peg2 = main.tile([128, 3], F32, tag="peg2")
        t3d = main.tile([128, 3], F32, tag="t3d")
        V.tensor_mul(ap3(prod9[:], 0, [[3, 3], [1, 3]]),
                     ap3(Fbc[:], 0, [[3, 3], [1, 3]]),
                     ap3(PEnd32[:], 0, [[0, 3], [1, 3]]))
        V.tensor_add(t3d[:, 0:3], ap3(prod9[:], 0, [[3, 3]]),
                     ap3(prod9[:], 1, [[3, 3]]))
        V.tensor_add(peg2[:, 0:3], t3d[:, 0:3],
                     ap3(prod9[:], 2, [[3, 3]]))
        nc.tensor.matmul(PSi[0:128, 0:3], tri[0:128, 0:128],
                         peg2[:, 0:3], start=True, stop=True)
        nc.scalar.copy(Pincb[0:128, 0:3], PSi[0:128, 0:3])

        # ---------------- mask ----------------

        # ------------- fused frame-fix + P_inc + mask + store -------------
        # coordinate 2's chain runs on Pool (STT ~1.5ns/elem) in parallel
        # with DVE doing coordinates 0 and 1.
        # coordinate 2's 3-op fix chain runs on Pool in parallel with DVE
        # doing coordinates 0/1; every coordinate's final masked store runs
        # on DVE, and each output DMA is issued from its own queue as soon
        # as its coordinate is done.
        # per-coordinate pipeline: fix -> masked store -> DMA, so each
        # output transfer is in flight while the next coordinate computes.
        # c=2 runs on Pool (emitted first so it starts right after the
        # scan), c=0/1 on DVE.
        # c=2's three products run on ACT (scalar.mul with per-partition
        # scale) while DVE does c=0/1; DVE only sums + masks c=2.
        p20 = tmps.tile([128, W], F16, tag="p20")
        p21 = tmps.tile([128, W], F16, tag="p21")
        p22 = tmps.tile([128, W], F16, tag="p22")
        nc.scalar.mul(p20[:], pv[:, 0:W], Fbc[:, 6:7])
        nc.scalar.mul(p21[:], pv[:, W:2 * W], Fbc[:, 7:8])
        nc.scalar.mul(p22[:], pv[:, 2 * W:3 * W], Fbc[:, 8:9])
        for c, q in ((0, nc.sync), (1, nc.scalar), (2, nc.sync)):
            x = tmps.tile([128, W], F16, tag="t1")
            if c == 2:
                V.tensor_add(x[:], p20[:], p21[:])
                V.tensor_add(x[:], x[:], p22[:])
            else:
                y = tmps.tile([128, W], F16, tag="t2")
                z = tmps.tile([128, W], F16, tag="t3")
                V.tensor_scalar_mul(x[:], pv[:, 0:W],
                                    Fbc[:, _e(c, 0):_e(c, 0) + 1])
                V.tensor_scalar_mul(y[:], pv[:, W:2 * W],
                                    Fbc[:, _e(c, 1):_e(c, 1) + 1])
                V.tensor_scalar_mul(z[:], pv[:, 2 * W:3 * W],
                                    Fbc[:, _e(c, 2):_e(c, 2) + 1])
                V.tensor_add(x[:], x[:], y[:])
                V.tensor_add(x[:], x[:], z[:])
            STT(Pmall[:, c * W:(c + 1) * W], x[:],
                Pincb[:, c:c + 1], maskp[:], op0=OP.add, op1=OP.mult)
            q.dma_start(out[:, c * W:(c + 1) * W],
                        Pmall[:, c * W:(c + 1) * W])


def _prep_alpha(input):
    # pure indexing: alphaN[r]=psi[r-1], alphaCA[r]=omega[r-1] (0 at r=0),
    # alphaC[r]=phi[r]; blocked (q, b, type, m) with residues 2-level
    # bit-reversed within each quarter: residue 4j+2b1+b0 -> col
    # 64*b0 + 32*b1 + j (makes every tree level's reads contiguous).
    phi, psi, om = input[:, 0], input[:, 1], input[:, 2]
    z1 = np.zeros((input.shape[0], 1), np.float32)
    aN = np.concatenate([z1, psi[:, :-1]], axis=1)
    aCA = np.concatenate([z1, om[:, :-1]], axis=1)
    alpha = np.stack([aCA, aN, phi], axis=1)          # [B, 3(A,N,C), 512]
    a = alpha.reshape(-1, 3, QN, NR // 4, 2, 2)       # [B,3,q,j,b1,b0]
    a = a.transpose(0, 2, 1, 5, 4, 3)                 # [B,q,3,b0,b1,j]
    return np.ascontiguousarray(a.reshape(-1, QN, 3, NR))


def _shard_alpha(alpha, i):
    sl = slice(i * BPC, (i + 1) * BPC)
    return np.ascontiguousarray(
        alpha[sl].transpose(1, 0, 2, 3)[::-1].reshape(QN * BPC, 3 * NR))


def _scalar_table(param):
    p = param.astype(np.float64)
    kap = p[[5, 1, 3]]
    Rl = p[[4, 0, 2]]
    ck, sk = np.cos(kap), np.sin(kap)
    t = np.zeros(24)
    t[0:6:2], t[1:6:2] = ck, sk
    t[6] = ck[0] * ck[1]
    t[7] = sk[0] * sk[1]
    t[8] = ck[0] * sk[1]
    t[9] = sk[0] * ck[1]
    t[10], t[11] = -t[9], -t[8]
    t[12], t[13], t[14], t[15] = -ck[0], -ck[1], -ck[2], -sk[1]
    t[16], t[17] = Rl[0] * ck[0], Rl[0] * sk[0]
    t[18], t[19] = Rl[1], Rl[2]
    return np.ascontiguousarray(
        np.broadcast_to(t.astype(np.float32), (128, 24)))


def _thr_table(alen_core):
    # partition row (3-q)*32 + b holds chain quarter q of protein b:
    # mask threshold = 3*len_b - q*W  (atom-coordinate count before cutoff)
    q = 3 - (np.arange(128) // BPC)
    b = np.arange(128) % BPC
    thr = 3.0 * alen_core[b] - q * W
    return np.ascontiguousarray(thr.astype(np.float32).reshape(128, 1))


def _get_nc():
    if "nc" not in _CACHE:
        _CACHE["nc"] = _build_graph()
    return _CACHE["nc"]


def kernel(input, param, angles_length, trace=False):
    input = np.ascontiguousarray(input, dtype=np.float32)
    param = np.ascontiguousarray(param, dtype=np.float32)
    angles_length = np.ascontiguousarray(angles_length, dtype=np.int32)
    nc = _get_nc()
    alpha = _prep_alpha(input)
    vbtab = _scalar_table(param)
    in_maps = []
    for i in range(NCORES):
        sl = slice(i * BPC, (i + 1) * BPC)
        in_maps.append({
            "input": _shard_alpha(alpha, i),
            "vbtab": vbtab,
            "thrv": _thr_table(angles_length[sl]),
        })
    res = run_bass_kernel_spmd(nc, in_maps, core_ids=list(range(NCORES)),
                               trace=trace)
    if trace:
        kernel._last_res = res
    outs = []
    for i in range(NCORES):
        r = res.results[i]["out"].astype(np.float32)  # [(q,b), (c,j)]
        r = r.reshape(QN, BPC, 3, W)[::-1]
        r = np.transpose(r, (1, 0, 3, 2)).reshape(BPC, 3 * QN * W)
        outs.append(r)
    full = np.concatenate(outs, axis=0).astype(np.float32)
    if trace:
        kernel._last_exec_ns = res.exec_time_ns
    return full


kernel._last_exec_ns = None

